# revision 6
# baseline (speedup 1.0000x reference)
"""AvgPool2d (kernel 2x2, stride 2) over x:(64,1024,1024) f32 -> (64,512,512).

Data-parallel across 8 NeuronCores: core c handles samples [8c, 8c+8).
Per core the shard is viewed as (1024, 8192): one "super-row" = 8 input
rows of one sample, so an SBUF tile [128, 8192] is exactly one sample
with partition p holding rows 8p..8p+7 (fully contiguous 4 MB DMA).

Compute per tile (DVE only; tensor_tensor_reduce would fuse the *0.25 but
crashes on HW in this runtime, so a separate tensor_scalar does it):
  stage 1 (vertical):   w[k][j]  = row(2k)[j] + row(2k+1)[j]        (tensor_tensor add)
  stage 2 (horizontal): o[k][j]  = w[k][2j] + w[k][2j+1]            (tensor_tensor add)
  stage 3 (mean):       o *= 0.25                                   (tensor_scalar, 2x mode)
Output tile [128, 2048] = one pooled sample, contiguous 1 MB DMA out.
Loads go on the SP HWDGE ring (nc.sync), stores on the ACT ring
(nc.scalar) so stores never head-of-line-block the next load.
"""

import sys

import numpy as np

_TRN_REPO = "/opt/trn_rl_repo"
if _TRN_REPO not in sys.path:
    sys.path.insert(0, _TRN_REPO)

N_CORES = 8
B, H, W = 64, 1024, 1024
PB = B // N_CORES          # samples per core
ROWS = PB * H // 8         # 1024 super-rows of 8 input rows
FD_IN = 8 * W              # 8192
OH, OW = H // 2, W // 2
FD_OUT = FD_IN // 4        # 2048
P = 128
TILES = ROWS // P          # 8 tiles per core, one sample each

_CACHE = {}


def build_nc():
    import concourse.mybir as mybir
    from concourse import bacc
    from concourse.tile import TileContext

    fp = mybir.dt.float32
    nc = bacc.Bacc("TRN2")
    x = nc.declare_dram_parameter("x", [ROWS, FD_IN], fp, isOutput=False)
    out = nc.declare_dram_parameter("out", [ROWS, FD_OUT], fp, isOutput=True)

    with TileContext(nc) as tc:
        with (
            tc.tile_pool(name="vin", bufs=3) as pin,
            tc.tile_pool(name="w", bufs=2) as pw,
            tc.tile_pool(name="o", bufs=3) as po,
        ):
            for t in range(TILES):
                v = pin.tile([P, FD_IN], fp)
                nc.sync.dma_start(out=v[:], in_=x[t * P : (t + 1) * P, :])

                w = pw.tile([P, FD_IN // 2], fp)
                v4 = v[:].rearrange("p (k two j) -> p k two j", k=4, two=2)
                w3 = w[:].rearrange("p (k j) -> p k j", k=4)
                nc.vector.tensor_tensor(
                    w3, v4[:, :, 0, :], v4[:, :, 1, :], mybir.AluOpType.add
                )

                o = po.tile([P, FD_OUT], fp)
                w4 = w[:].rearrange("p (k j two) -> p k j two", k=4, two=2)
                o3 = o[:].rearrange("p (k j) -> p k j", k=4)
                nc.vector.tensor_tensor(
                    o3, w4[:, :, :, 0], w4[:, :, :, 1], mybir.AluOpType.add
                )
                nc.vector.tensor_scalar_mul(o[:], o[:], 0.25)

                nc.scalar.dma_start(out=out[t * P : (t + 1) * P, :], in_=o[:])
    nc.compile()
    return nc


def _get_nc():
    if "nc" not in _CACHE:
        _CACHE["nc"] = build_nc()
    return _CACHE["nc"]


def kernel(**inputs) -> np.ndarray:
    from concourse.bass_utils import run_bass_kernel_spmd

    x = np.ascontiguousarray(np.asarray(inputs["x"], dtype=np.float32))
    assert x.shape == (B, H, W)

    nc = _get_nc()
    in_maps = [
        {"x": x[c * PB : (c + 1) * PB].reshape(ROWS, FD_IN)} for c in range(N_CORES)
    ]
    res = run_bass_kernel_spmd(nc, in_maps, core_ids=list(range(N_CORES))).results

    out = np.empty((B, OH, OW), np.float32)
    for c in range(N_CORES):
        out[c * PB : (c + 1) * PB] = res[c]["out"].reshape(PB, OH, OW)
    return out


# revision 7
# speedup vs baseline: 16.3518x; 16.3518x over previous
"""AvgPool2d (kernel 2x2, stride 2) over x:(64,1024,1024) f32 -> (64,512,512).

Data-parallel across 8 NeuronCores: core c handles samples [8c, 8c+8).
Per core the shard is viewed as (1024, 8192): one "super-row" = 8 input
rows of one sample, so an SBUF tile [128, 8192] is exactly one sample
with partition p holding rows 8p..8p+7 (fully contiguous 4 MB DMA).

Compute per tile (DVE only; tensor_tensor_reduce would fuse the *0.25 but
crashes on HW in this runtime, so a separate tensor_scalar does it):
  stage 1 (vertical):   w[k][j]  = row(2k)[j] + row(2k+1)[j]        (tensor_tensor add)
  stage 2 (horizontal): o[k][j]  = w[k][2j] + w[k][2j+1]            (tensor_tensor add)
  stage 3 (mean):       o *= 0.25                                   (tensor_scalar, 2x mode)
Output tile [128, 2048] = one pooled sample, contiguous 1 MB DMA out.
Loads go on the SP HWDGE ring (nc.sync), stores on the ACT ring
(nc.scalar) so stores never head-of-line-block the next load.
"""

import sys

import numpy as np

_TRN_REPO = "/opt/trn_rl_repo"
if _TRN_REPO not in sys.path:
    sys.path.insert(0, _TRN_REPO)

N_CORES = 8
B, H, W = 64, 1024, 1024
PB = B // N_CORES          # samples per core
ROWS = PB * H // 8         # 1024 super-rows of 8 input rows
FD_IN = 8 * W              # 8192
OH, OW = H // 2, W // 2
FD_OUT = FD_IN // 4        # 2048
P = 128
TILES = ROWS // P          # 8 tiles per core, one sample each

_CACHE = {}


def build_nc(repeat: int = 1):
    """repeat>1 re-runs the whole pooling pass inside one NEFF; used by
    test.py to measure per-pass HW time as a slope (dispatch cancels)."""
    import concourse.mybir as mybir
    from concourse import bacc
    from concourse.tile import TileContext

    fp = mybir.dt.float32
    nc = bacc.Bacc("TRN2")
    x = nc.declare_dram_parameter("x", [ROWS, FD_IN], fp, isOutput=False)
    out = nc.declare_dram_parameter("out", [ROWS, FD_OUT], fp, isOutput=True)

    with TileContext(nc) as tc:
        with (
            tc.tile_pool(name="vin", bufs=3) as pin,
            tc.tile_pool(name="w", bufs=2) as pw,
            tc.tile_pool(name="o", bufs=3) as po,
        ):
            for t in [t for _ in range(repeat) for t in range(TILES)]:
                v = pin.tile([P, FD_IN], fp)
                nc.sync.dma_start(out=v[:], in_=x[t * P : (t + 1) * P, :])

                w = pw.tile([P, FD_IN // 2], fp)
                v4 = v[:].rearrange("p (k two j) -> p k two j", k=4, two=2)
                w3 = w[:].rearrange("p (k j) -> p k j", k=4)
                nc.vector.tensor_tensor(
                    w3, v4[:, :, 0, :], v4[:, :, 1, :], mybir.AluOpType.add
                )

                o = po.tile([P, FD_OUT], fp)
                w4 = w[:].rearrange("p (k j two) -> p k j two", k=4, two=2)
                o3 = o[:].rearrange("p (k j) -> p k j", k=4)
                nc.vector.tensor_tensor(
                    o3, w4[:, :, :, 0], w4[:, :, :, 1], mybir.AluOpType.add
                )
                nc.vector.tensor_scalar_mul(o[:], o[:], 0.25)

                nc.scalar.dma_start(out=out[t * P : (t + 1) * P, :], in_=o[:])
    nc.compile()
    return nc


def _get_nc():
    if "nc" not in _CACHE:
        _CACHE["nc"] = build_nc()
    return _CACHE["nc"]


def kernel(**inputs) -> np.ndarray:
    from concourse.bass_utils import run_bass_kernel_spmd

    x = np.ascontiguousarray(np.asarray(inputs["x"], dtype=np.float32))
    assert x.shape == (B, H, W)

    nc = _get_nc()
    in_maps = [
        {"x": x[c * PB : (c + 1) * PB].reshape(ROWS, FD_IN)} for c in range(N_CORES)
    ]
    res = run_bass_kernel_spmd(nc, in_maps, core_ids=list(range(N_CORES))).results

    out = np.empty((B, OH, OW), np.float32)
    for c in range(N_CORES):
        out[c * PB : (c + 1) * PB] = res[c]["out"].reshape(PB, OH, OW)
    return out
